# revision 18
# baseline (speedup 1.0000x reference)
"""EvolveGCN-O on 8 Trainium2 NeuronCores (Bass/Tile), v2.

Algebraic reduction (same as v1): only the last scan step's h2 reaches the
MLP, and the mat-GRU weight evolution is data-independent, so
    W1_T, W2_T = mat_gru^T(W1_0), mat_gru^T(W2_0)
    out = relu((b*(A (a*X) W1 ... )) ...)    with  norm[e] = a[src]*b[dst].
Additionally v2 uses A(X W1) = (A X) W1: the layer-1 gather table is the
host-prescaled  a*X  in bf16, shipped as a plain input to every core --
no on-device X processing and no X AllGather; layer-1 gathers start at t~0
and overlap the serial GRU weight evolution.

Aggregation (per core, nodes/edges sharded by dst):
  - Single gather table [51200, 128] bf16 covering all 8 cores' rows
    (AB halves so the h1 AllGather can run as two overlapped halves).
    dma_gather indices are int16 with the table base offset to row 32768:
    signed idx = row - 32768 covers the full table.  Only a TRAILING run
    of negative indices is skipped by the ucode, so host prep guarantees
    the last slot of every 2048-index span has a non-negative index.
  - Edges of each dst-chunk (128 nodes) are laid out by *rank*: the g-th
    edge of each node sits at its node's slot in identity-block g, so the
    segment-sum is a matmul with a constant identity stationary -- no
    per-group one-hot build on the Vector engine.  Rank >= R_ID edges go
    to classic one-hot tail blocks (bf16 is_equal builds, 2x DVE mode).
  - Gather spans of 2048 indices rotate across 4 SWDGE queues: queue
    descriptor-generation workers run in parallel (~2ns/idx aggregate vs
    ~8ns/idx serialized).
"""

import sys
import numpy as np

for _p in ('/opt/trn_rl_repo', '/root/.axon_site'):
    if _p not in sys.path:
        sys.path.insert(0, _p)

import os
NCORES = 8
SPAN = int(os.environ.get("K_SPAN", "1024"))  # idxs per dma_gather
NQ = 4                                         # SWDGE queues (ucode max)
PHASES = int(os.environ.get("K_PHASES", "4"))  # 1=GRU 2=+L1 3=+AG 4=full
NOAG = os.environ.get("K_NOAG", "0") == "1"
N_DVE = int(os.environ.get("K_NDVE", "0"))     # id-ranks accumulated on DVE
R_ID = int(os.environ.get("K_RID", "16"))
GBUFS = int(os.environ.get("K_GBUFS", "36"))
RRELU_SLOPE = (1.0 / 8.0 + 1.0 / 3.0) / 2.0

N = 50000
NPC = N // NCORES            # 6250
PADNPC = 6400
NCH = PADNPC // 128          # 50
HALF = PADNPC // 2           # 3200 rows per AB half
NTAB = NCORES * PADNPC       # 51200
BASE = 32768                 # gather base row (signed idx = row - BASE)
PAD_ROW0 = NCORES * HALF + 7 * HALF + (NPC - HALF)   # 51050, first zero row
NPADROWS = PADNPC - NPC      # 150 zero rows (core 7 B-half tail)

_CACHE = {}


PART_L = np.array([0, 1664, 3328, 4992, 6272, 6400])  # local-row part bounds
PART_ROWS = np.diff(PART_L)                          # rows per part per core
PART_BASE = np.concatenate(([0], np.cumsum(PART_ROWS * NCORES)[:-1]))


def _row_of(n):
    """Node id -> gather-table row (4 AllGather parts layout)."""
    k = n // NPC
    l = n - k * NPC
    p = np.searchsorted(PART_L, l, side='right') - 1
    return PART_BASE[p] + k * PART_ROWS[p] + (l - PART_L[p])


# ----------------------------------------------------------------------------
# host-side graph prep
# ----------------------------------------------------------------------------

def _prep(src, dst):
    E = src.shape[0]
    core = dst // NPC
    l = dst - core * NPC
    chunk = l // 128
    slot = l % 128
    row_src = _row_of(src)

    # rank of each edge within its (core, chunk, slot)
    key = (core * NCH + chunk) * 128 + slot
    order = np.lexsort((src, key))
    sk = key[order]
    starts = np.searchsorted(sk, np.arange(NCORES * NCH * 128))
    rank_sorted = np.arange(E) - starts[sk]
    rank = np.empty(E, np.int64)
    rank[order] = rank_sorted

    deg_per_slot = np.bincount(key, minlength=NCORES * NCH * 128)

    # pick R_ID minimizing total gathered slots
    if R_ID > 0:
        rid = R_ID
    else:
        best = None
        dps = deg_per_slot.reshape(NCORES, NCH, 128)
        for r in range(4, 26):
            idb = r * NCH * 128
            tail_cnt = np.maximum(dps - r, 0).sum(axis=2)        # [NCORES, NCH]
            gtl = np.maximum(1, -(-(tail_cnt.max(axis=0) + 8) // 128))
            tot = idb + gtl.sum() * 128
            if best is None or tot < best[0]:
                best = (tot, r)
        rid = best[1]

    is_tail = rank >= rid
    tail_cnt = np.bincount((core * NCH + chunk)[is_tail],
                           minlength=NCORES * NCH).reshape(NCORES, NCH)
    g_tl = np.maximum(1, -(-(tail_cnt.max(axis=0) + 8) // 128))  # [NCH]

    blocks_per_chunk = rid + g_tl
    off_blocks = np.concatenate(([0], np.cumsum(blocks_per_chunk)[:-1]))
    nblk_real = int(blocks_per_chunk.sum())
    nspan = -(-nblk_real // (SPAN // 128))
    nblk = nspan * (SPAN // 128)
    L = nblk * 128

    tg_off = np.concatenate(([0], np.cumsum(g_tl)[:-1]))
    NTG = int(g_tl.sum())

    # index stream (per core) + tail slot values
    padvals = PAD_ROW0 + (np.arange(L) % NPADROWS)
    idx = np.tile(padvals, (NCORES, 1))                       # [NCORES, L]
    ds = np.full((NCORES, NTG, 128), -2.0, np.float32)

    idm = ~is_tail
    pos_id = (off_blocks[chunk[idm]] + rank[idm]) * 128 + slot[idm]
    idx[core[idm], pos_id] = row_src[idm]

    # tails: running index per (core, chunk) in lexsort order
    tmask_sorted = rank_sorted >= rid
    t_order = order[tmask_sorted]                 # edge ids, grouped by (core,chunk)
    t_key = (core * NCH + chunk)[t_order]
    t_starts = np.searchsorted(t_key, np.arange(NCORES * NCH))
    jj = np.arange(len(t_order)) - t_starts[t_key]
    t_chunk = chunk[t_order]
    t_core = core[t_order]
    pos_t = (off_blocks[t_chunk] + rid + jj // 128) * 128 + (jj % 128)
    idx[t_core, pos_t] = row_src[t_order]
    ds[t_core, tg_off[t_chunk] + jj // 128, jj % 128] = slot[t_order]

    # span-boundary guard: last idx of every span must be >= BASE (trailing
    # negative int16 indices are skipped by the gather ucode)
    col_of = {}          # block -> ds column, for tail blocks
    for c in range(NCH):
        for j in range(g_tl[c]):
            col_of[off_blocks[c] + rid + j] = tg_off[c] + j
    blk_chunk = np.full(nblk, -1, np.int64)
    for c in range(NCH):
        blk_chunk[off_blocks[c]:off_blocks[c] + blocks_per_chunk[c]] = c
    for s in range(nspan):
        pb = SPAN * (s + 1) - 1
        blk = pb // 128
        c = blk_chunk[blk] if blk < nblk_real else -1
        if c < 0:
            continue
        for k in range(NCORES):
            v = idx[k, pb]
            if v >= BASE:
                continue
            # find a pad slot in chunk c's tail region (not at a boundary)
            t0 = (off_blocks[c] + rid) * 128
            t1 = t0 + g_tl[c] * 128
            tr = np.arange(t0, t1)
            tcols = tg_off[c] + (tr - t0) // 128
            ispad = (ds[k, tcols, tr % 128] < 0) & (tr != pb) & \
                    ((tr % SPAN) != SPAN - 1)
            cand = tr[ispad]
            assert len(cand) > 0, "no pad slot for span guard"
            tp = cand[-1]
            tpc = tg_off[c] + (tp - t0) // 128
            b_in = blk - off_blocks[c]
            if b_in < rid:
                # demote identity edge (slot 127, rank b_in) to tail
                idx[k, tp] = v
                ds[k, tpc, tp % 128] = 127.0
                idx[k, pb] = padvals[pb]
            else:
                # swap tail entry with the pad
                pbc = col_of[blk]
                idx[k, tp] = v
                ds[k, tpc, tp % 128] = ds[k, pbc, pb % 128]
                idx[k, pb] = padvals[pb]
                ds[k, pbc, pb % 128] = -2.0
    assert (idx[:, SPAN - 1::SPAN] >= BASE).all()

    # host-built tail one-hot matrices [NCORES, 128(edge), NTG*128(slotcols)]
    oh = np.zeros((NCORES, 128, NTG * 128), np.float32)
    kk, col, pp = np.nonzero(ds >= 0)
    oh[kk, pp, col * 128 + ds[kk, col, pp].astype(np.int64)] = 1.0
    oh_w = oh.astype('bfloat16')

    # wrapped layouts
    idx16 = (idx - BASE).astype(np.int16)
    idx_w = np.ascontiguousarray(
        np.tile(idx16.reshape(NCORES, L // 16, 16).transpose(0, 2, 1), (1, 8, 1)))
    ds_w = np.ascontiguousarray(ds.transpose(0, 2, 1))

    deg_out = np.bincount(src, minlength=N).astype(np.int32)
    deg_in = np.bincount(dst, minlength=N).astype(np.int32)

    def wrap_nodevec(v):   # [N] -> [NCORES, 128, NCH]
        out = np.zeros((NCORES, 128, NCH), v.dtype)
        for k in range(NCORES):
            sp = np.zeros(PADNPC, v.dtype)
            sp[:NPC] = v[k * NPC:(k + 1) * NPC]
            out[k] = sp.reshape(NCH, 128).T
        return np.ascontiguousarray(out)

    meta = dict(E=E, rid=rid, g_tl=g_tl, off_blocks=off_blocks,
                tg_off=tg_off, NTG=NTG, L=L, nspan=nspan, nblk_real=nblk_real,
                deg_out=deg_out)
    return meta, idx_w, oh_w, wrap_nodevec(deg_in), wrap_nodevec(deg_out)


# ----------------------------------------------------------------------------
# device program
# ----------------------------------------------------------------------------

def _build(meta, T):
    import concourse.bass as bass
    import concourse.bacc as bacc
    import concourse.mybir as mybir
    import concourse.tile as tile

    f32 = mybir.dt.float32
    bf16 = mybir.dt.bfloat16
    i16 = mybir.dt.int16
    i32 = mybir.dt.int32
    AF = mybir.ActivationFunctionType
    OP = mybir.AluOpType

    rid = meta['rid']
    g_tl = meta['g_tl']
    off_blocks = meta['off_blocks']
    tg_off = meta['tg_off']
    NTG = meta['NTG']
    L = meta['L']
    nspan = meta['nspan']

    nc = bacc.Bacc(None, target_bir_lowering=False, num_swdge_queues=NQ)

    dp = nc.declare_dram_parameter
    t1 = dp("t1", [NTAB, 128], bf16, isOutput=False)
    W1_0 = dp("W1_0", [128, 128], f32, isOutput=False)
    W2_0 = dp("W2_0", [128, 127], f32, isOutput=False)
    g1_WT = dp("g1_WT", [3, 128, 128], f32, isOutput=False)
    g1_UT = dp("g1_UT", [3, 128, 128], f32, isOutput=False)
    g1_b = dp("g1_b", [3, 128, 128], f32, isOutput=False)
    g2_WT = dp("g2_WT", [3, 128, 128], f32, isOutput=False)
    g2_UT = dp("g2_UT", [3, 128, 128], f32, isOutput=False)
    g2_b = dp("g2_b", [3, 128, 127], f32, isOutput=False)
    mw1 = dp("mw1", [127, 64], f32, isOutput=False)
    mb1 = dp("mb1", [64, 1], f32, isOutput=False)
    mw2 = dp("mw2", [64, 2], f32, isOutput=False)
    mb2 = dp("mb2", [1, 2], f32, isOutput=False)
    idx_d = dp("idx", [128, L // 16], i16, isOutput=False)
    oh_d = dp("oh", [128, NTG * 128], bf16, isOutput=False)
    din_d = dp("din", [128, NCH], i32, isOutput=False)
    dout_d = dp("dout", [128, NCH], i32, isOutput=False)
    outk = dp("outk", [2, PADNPC], f32, isOutput=True)

    h1x = nc.dram_tensor("h1x", [NTAB, 128], bf16, addr_space="Shared")
    rg = [list(range(NCORES))]

    with tile.TileContext(nc) as tc:
        with tc.tile_pool(name="const", bufs=1) as cp, \
             tc.tile_pool(name="work", bufs=4) as wp, \
             tc.tile_pool(name="gpool", bufs=GBUFS) as gp, \
             tc.tile_pool(name="ps", bufs=1, space="PSUM") as pp, \
             tc.tile_pool(name="psg", bufs=2, space="PSUM") as pgru, \
             tc.tile_pool(name="psacc", bufs=2, space="PSUM") as pacc, \
             tc.tile_pool(name="dram", bufs=1, space="DRAM") as dr:

            sync, vec, act, pe, gps = nc.sync, nc.vector, nc.scalar, nc.tensor, nc.gpsimd

            _ctr = [0]

            def ctile(shape, dt):
                _ctr[0] += 1
                return cp.tile(shape, dt, tag=f"c{_ctr[0]}", name=f"c{_ctr[0]}")

            def load(shape, dt, src_ap, pool=None, tag=None):
                t = ctile(shape, dt) if pool is None else pool.tile(shape, dt, tag=tag)
                sync.dma_start(out=t[:], in_=src_ap)
                return t

            # ---------------- constants ----------------
            iota_f = cp.tile([128, 128], f32, tag="iota_f")
            gps.iota(iota_f[:], pattern=[[1, 128]], base=0, channel_multiplier=0,
                     allow_small_or_imprecise_dtypes=True)
            iota_c = cp.tile([128, 1], f32, tag="iota_c")
            gps.iota(iota_c[:], pattern=[[1, 1]], base=0, channel_multiplier=1,
                     allow_small_or_imprecise_dtypes=True)
            ident = cp.tile([128, 128], f32, tag="ident")
            vec.tensor_scalar(ident[:], iota_f[:], iota_c[:, 0:1], None, OP.is_equal)
            ident_b = ctile([128, 128], bf16)
            vec.tensor_copy(ident_b[:], ident[:])
            iota_b = ctile([128, 128], bf16)
            vec.tensor_copy(iota_b[:], iota_f[:])

            idx_t = cp.tile([128, L // 16], i16, tag="idxt")
            _c0 = (L // 16) // 8
            sync.dma_start(out=idx_t[:, :_c0], in_=idx_d[:, :_c0])
            sync.dma_start(out=idx_t[:, _c0:], in_=idx_d[:, _c0:])
            oh_t = load([128, NTG * 128], bf16, oh_d[:])

            def rsqrt_vec(dsrc):
                it = wp.tile([128, NCH], i32, tag="deg")
                sync.dma_start(out=it[:], in_=dsrc[:])
                ft = wp.tile([128, NCH], f32, tag="degf")
                vec.tensor_copy(ft[:], it[:])
                vec.tensor_scalar(ft[:], ft[:], 1.0, None, OP.max)
                st = wp.tile([128, NCH], f32, tag="degs")
                act.activation(st[:], ft[:], AF.Sqrt)
                ot = ctile([128, NCH], f32)
                vec.reciprocal(ot[:], st[:])
                return ot

            b_loc = rsqrt_vec(din_d)
            a_loc = rsqrt_vec(dout_d)
            ab_loc = ctile([128, NCH], f32)
            vec.tensor_tensor(ab_loc[:], a_loc[:], b_loc[:], OP.mult)

            w1m_t = load([127, 64], f32, mw1[:])
            b1c = load([64, 1], f32, mb1[:])
            w2m_t = load([64, 2], f32, mw2[:])
            b2r = load([1, 2], f32, mb2[:])
            ones1 = ctile([1, 128], f32)
            vec.memset(ones1[:], 1.0)

            # ---------------- GRU weight evolution ----------------
            def gru_cell(gWT, gUT, gB, W0, width, tag):
                gz = ctile([128, 128], f32)
                gr = ctile([128, 128], f32)
                gw2 = load([128, 128], f32, gWT[2])
                gu2 = load([128, 128], f32, gUT[2])
                t0 = load([128, 128], f32, gWT[0], pool=wp, tag="gl")
                t1_ = load([128, 128], f32, gUT[0], pool=wp, tag="gl")
                vec.tensor_tensor(gz[:], t0[:], t1_[:], OP.add)
                t2 = load([128, 128], f32, gWT[1], pool=wp, tag="gl")
                t3 = load([128, 128], f32, gUT[1], pool=wp, tag="gl")
                vec.tensor_tensor(gr[:], t2[:], t3[:], OP.add)
                bz = load([128, width], f32, gB[0])
                br = load([128, width], f32, gB[1])
                bh = load([128, width], f32, gB[2])
                Wst = load([128, width], f32, W0[:])
                for _ in range(T):
                    zp = pgru.tile([128, width], f32, tag="g")
                    pe.matmul(zp[:], gz[:], Wst[:], start=True, stop=True)
                    zs = wp.tile([128, width], f32, tag=tag + "zs")
                    vec.tensor_tensor(zs[:], zp[:], bz[:], OP.add)
                    act.activation(zs[:], zs[:], AF.Sigmoid)
                    rp = pgru.tile([128, width], f32, tag="g")
                    pe.matmul(rp[:], gr[:], Wst[:], start=True, stop=True)
                    rs = wp.tile([128, width], f32, tag=tag + "rs")
                    vec.tensor_tensor(rs[:], rp[:], br[:], OP.add)
                    act.activation(rs[:], rs[:], AF.Sigmoid)
                    rW = wp.tile([128, width], f32, tag=tag + "rw")
                    vec.tensor_tensor(rW[:], rs[:], Wst[:], OP.mult)
                    hp = pgru.tile([128, width], f32, tag="g")
                    pe.matmul(hp[:], gw2[:], Wst[:], start=True, stop=False)
                    pe.matmul(hp[:], gu2[:], rW[:], start=False, stop=True)
                    hs = wp.tile([128, width], f32, tag=tag + "hs")
                    vec.tensor_tensor(hs[:], hp[:], bh[:], OP.add)
                    act.activation(hs[:], hs[:], AF.Tanh)
                    vec.tensor_tensor(hs[:], hs[:], Wst[:], OP.subtract)
                    vec.tensor_tensor(hs[:], zs[:], hs[:], OP.mult)
                    Wn = wp.tile([128, width], f32, tag=tag + "wn")
                    vec.tensor_tensor(Wn[:], Wst[:], hs[:], OP.add)
                    Wst = Wn
                return Wst

            W1f = gru_cell(g1_WT, g1_UT, g1_b, W1_0, 128, "c1")
            W2f = gru_cell(g2_WT, g2_UT, g2_b, W2_0, 127, "c2")

            W1b = ctile([128, 128], bf16)
            vec.tensor_copy(W1b[:], W1f[:])
            # C1 = W2f @ mlp_w1  via lhsT = W2f^T
            tps = pgru.tile([128, 128], f32, tag="g")
            pe.transpose(tps[:127, :], W2f[:], ident[:])
            W2T = wp.tile([127, 128], f32, tag="w2t")
            vec.tensor_copy(W2T[:], tps[:127, :])
            wps = pgru.tile([128, 64], f32, tag="g")
            pe.matmul(wps[:], W2T[:], w1m_t[:], start=True, stop=True)
            c1b = ctile([128, 64], bf16)
            vec.tensor_copy(c1b[:], wps[:])
            w2m_b = ctile([64, 2], bf16)
            vec.tensor_copy(w2m_b[:], w2m_t[:])
            b2r_b = ctile([1, 2], bf16)
            vec.tensor_copy(b2r_b[:], b2r[:])
            ones_b = ctile([1, 128], bf16)
            vec.memset(ones_b[:], 1.0)

            # ---------------- aggregation layers ----------------
            h1slab = cp.tile([128, NCH, 128], bf16, tag="h1slab")
            outslab = cp.tile([2, NCH, 128], f32, tag="outslab")
            vec.memset(outslab[:], 0.0)
            h1loc = dr.tile([PADNPC, 128], bf16)

            def run_layer(table, epilogue, after_chunk):
                gtiles = [None] * nspan

                def need_span(s):
                    if gtiles[s] is None:
                        gt = gp.tile([128, SPAN // 128, 128], bf16, tag="g")
                        gps.dma_gather(
                            gt[:], table, idx_t[:, s * (SPAN // 16):(s + 1) * (SPAN // 16)],
                            SPAN, SPAN, 128, single_packet=False, queue_num=s % NQ)
                        gtiles[s] = gt
                    return gtiles[s]

                for c in range(NCH):
                    nb = rid + g_tl[c]
                    acc = pacc.tile([128, 128], f32, tag="acc")
                    zdve = None
                    ndve = min(N_DVE, rid - 1) if N_DVE > 0 else 0
                    first_pe = True
                    for b in range(nb):
                        blk = off_blocks[c] + b
                        gt = need_span(blk // (SPAN // 128))
                        gv = gt[:, blk % (SPAN // 128), :]
                        if b < rid:
                            if b < ndve:
                                if zdve is None:
                                    zdve = wp.tile([128, 128], f32, tag="zdve")
                                    vec.tensor_copy(zdve[:], gv)
                                else:
                                    vec.tensor_tensor(zdve[:], zdve[:], gv, OP.add)
                                continue
                            st = ident_b
                        else:
                            j = tg_off[c] + (b - rid)
                            st = oh_t[:, j * 128:(j + 1) * 128]
                        pe.matmul(acc[:], st if isinstance(st, type(gv)) else st[:], gv, start=first_pe, stop=(b == nb - 1))
                        first_pe = False
                    epilogue(c, acc, zdve)
                    if after_chunk is not None:
                        after_chunk(c)

            def epi1(c, acc, zdve):
                # Z1 chunk -> h1 table row values: ab*rrelu(Z1@W1) = Lrelu(ab*(Z1@W1))
                z = wp.tile([128, 128], bf16, tag="e1z")
                if zdve is not None:
                    vec.tensor_tensor(z[:], acc[:], zdve[:], OP.add)
                else:
                    vec.tensor_copy(z[:], acc[:])
                tp = pp.tile([128, 128], bf16, tag="tpb")
                pe.transpose(tp[:], z[:], ident_b[:])
                zT = wp.tile([128, 128], bf16, tag="e1t")
                vec.tensor_copy(zT[:], tp[:])
                tmm = pp.tile([128, 128], f32, tag="mm1")
                pe.matmul(tmm[:], zT[:], W1b[:], start=True, stop=True)
                act.activation(h1slab[:, c, :], tmm[:], AF.Prelu,
                               scale=ab_loc[:, c:c + 1], alpha=RRELU_SLOPE)

            part_lc = [int(x) // 128 for x in PART_L]      # chunk bounds

            def stream_h1(c):
                if c + 1 not in part_lc[1:]:
                    return
                part = part_lc[1:].index(c + 1)
                lo_c, hi_c = part_lc[part], part_lc[part + 1]
                lo_r, hi_r = lo_c * 128, hi_c * 128
                sync.dma_start(
                    out=h1loc[lo_r:hi_r].rearrange("(b p) e -> p b e", p=128),
                    in_=h1slab[:, lo_c:hi_c, :])
                if not NOAG and PHASES >= 3:
                    base = int(PART_BASE[part])
                    nrows = NCORES * (hi_r - lo_r)
                    gps.collective_compute(
                        "AllGather", mybir.AluOpType.bypass, replica_groups=rg,
                        ins=[h1loc[lo_r:hi_r].opt()],
                        outs=[h1x[base:base + nrows].opt()])

            def epi2(c, acc, zdve):
                zb = wp.tile([128, 128], bf16, tag="e2z")
                if zdve is not None:
                    t0 = wp.tile([128, 128], f32, tag="e2m")
                    vec.tensor_tensor(t0[:], acc[:], zdve[:], OP.add)
                    act.activation(zb[:], t0[:], AF.Identity,
                                   scale=b_loc[:, c:c + 1])
                else:
                    act.activation(zb[:], acc[:], AF.Identity,
                                   scale=b_loc[:, c:c + 1])
                tp = pp.tile([128, 128], bf16, tag="tpb")
                pe.transpose(tp[:], zb[:], ident_b[:])
                zT = wp.tile([128, 128], bf16, tag="e2t")
                vec.tensor_copy(zT[:], tp[:])
                u = pp.tile([64, 128], f32, tag="u")
                pe.matmul(u[:], c1b[:], zT[:], start=True, stop=True)
                ur = wp.tile([64, 128], bf16, tag="ur")
                act.activation(ur[:], u[:], AF.Relu, bias=b1c[:, 0:1])
                o = pp.tile([2, 128], f32, tag="o")
                pe.matmul(o[:], w2m_b[:], ur[:], start=True, stop=False)
                pe.matmul(o[:], b2r_b[:], ones_b[:], start=False, stop=True)
                vec.tensor_copy(outslab[:, c, :], o[:])

            if PHASES >= 2:
                run_layer(t1[BASE:, :], epi1, stream_h1)
            if PHASES >= 4:
                run_layer(h1x[BASE:, :], epi2, None)
            sync.dma_start(out=outk[:].rearrange("o (b p) -> o b p", p=128),
                           in_=outslab[:])

    nc.finalize()
    return nc


# ----------------------------------------------------------------------------
# entry points
# ----------------------------------------------------------------------------

def _get_compiled(inputs):
    feats = np.asarray(inputs["feats"], np.float32)
    src = np.asarray(inputs["src"])
    dst = np.asarray(inputs["dst"])
    T = feats.shape[0]
    key = (T, N, src.shape[0], int(src[0]), int(dst[0]), int(src[-1]),
           PHASES, NOAG, N_DVE, R_ID, GBUFS)
    if key in _CACHE:
        nc, meta, idx_w, ds_w, din_w, dout_w = _CACHE[key]
    else:
        meta, idx_w, ds_w, din_w, dout_w = _prep(src, dst)
        nc = _build(meta, T)
        _CACHE[key] = (nc, meta, idx_w, ds_w, din_w, dout_w)

    # layer-1 gather table: a-scaled last-step features, AB row layout
    x7 = feats[T - 1]
    a = 1.0 / np.sqrt(np.maximum(meta['deg_out'], 1.0).astype(np.float32))
    t1_full = np.zeros((NTAB, 128), np.float32)
    rows = _row_of(np.arange(N))
    t1_full[rows] = x7 * a[:, None]
    t1_full = t1_full.astype('bfloat16')

    in_maps = []
    for k in range(NCORES):
        m = {
            "t1": t1_full,
            "W1_0": np.asarray(inputs["W1_0"], np.float32),
            "W2_0": np.asarray(inputs["W2_0"], np.float32),
            "g1_WT": np.ascontiguousarray(np.asarray(inputs["g1_W"], np.float32).transpose(0, 2, 1)),
            "g1_UT": np.ascontiguousarray(np.asarray(inputs["g1_U"], np.float32).transpose(0, 2, 1)),
            "g1_b": np.asarray(inputs["g1_b"], np.float32),
            "g2_WT": np.ascontiguousarray(np.asarray(inputs["g2_W"], np.float32).transpose(0, 2, 1)),
            "g2_UT": np.ascontiguousarray(np.asarray(inputs["g2_U"], np.float32).transpose(0, 2, 1)),
            "g2_b": np.asarray(inputs["g2_b"], np.float32),
            "mw1": np.asarray(inputs["mlp_w1"], np.float32),
            "mb1": np.asarray(inputs["mlp_b1"], np.float32).reshape(64, 1),
            "mw2": np.asarray(inputs["mlp_w2"], np.float32),
            "mb2": np.asarray(inputs["mlp_b2"], np.float32).reshape(1, 2),
            "idx": idx_w[k], "oh": ds_w[k],
            "din": din_w[k], "dout": dout_w[k],
        }
        in_maps.append(m)
    return nc, in_maps, dict(meta, N=N, NPC=NPC)


def _install_ntff_hook():
    import types
    try:
        import antenv
        if "antenv.axon_hooks" not in sys.modules:
            m = types.ModuleType("antenv.axon_hooks")
            h = [None]
            m.set_axon_ntff_profile_hook = lambda x: h.__setitem__(0, x)
            m.get_axon_ntff_profile_hook = lambda: h[0]
            sys.modules["antenv.axon_hooks"] = m
            antenv.axon_hooks = m
            from trn_agent_boot.trn_boot import _ntff_profile_via_ctypes
            m.set_axon_ntff_profile_hook(
                _ntff_profile_via_ctypes('/opt/axon/libaxon_pjrt.so'))
    except Exception:
        pass


def kernel(**inputs):
    from concourse.bass_utils import run_bass_kernel_spmd
    _install_ntff_hook()
    nc, in_maps, meta = _get_compiled(inputs)
    res = run_bass_kernel_spmd(nc, in_maps, list(range(NCORES)))
    out = np.empty((N, 2), np.float32)
    for k in range(NCORES):
        out[k * NPC:(k + 1) * NPC] = res.results[k]["outk"][:, :NPC].T
    return out
